# revision 24
# baseline (speedup 1.0000x reference)
"""Tensor-parallel MHA prefill kernel for 8 TRN2 NeuronCores.

Sharding: heads across cores (4 Q heads + 1 KV head per core).
Per core, fully pipelined by 512-row sequence chunks:
  - QKV computed TRANSPOSED (weights-stationary matmuls, out [feat, seq])
    so scores need no on-chip q/k transposes; V is transposed back per
    128-tile on the PE.
  - RoPE via a pair-swap permutation matmul on the PE plus two vector
    multiplies against host-expanded cos/sin rows (V rides along with
    cos=1/sin=0 rows).
  - Attention in scores-transposed orientation, softmax denominators via
    an appended ones-column, AV accumulated directly in PSUM chains.
  - Striped AllToAll: each (chunk, head-pair) ships [8, 128, 64] rows as
    its own small collective, overlapped with later chunks.
  - Output projection split in halves that overlap the last attention
    chunk and the final collective.
Host only slices/transposes/casts weights and the input, and reorders
the 8 cores' 64-row output stripes.
"""
import os
import numpy as np
import ml_dtypes

N_CORES = 8
S = 2048
D = 2048
NH = 32
HD = 64
HPC = NH // N_CORES      # 4 q heads per core
QW = HPC * HD            # 256
QKV = QW + 2 * HD        # 384
SCALE = 1.0 / np.sqrt(HD)

IC = 512                 # q-chunk width (and kv j-chunk per sc)
NCH = S // IC            # 4 chunks
NS = S // 128            # 16 j-tiles
HD1 = HD + 1             # 65
NWT = 3                  # weight tiles: {q0q1, q2q3, k|v}
ND = D // 128            # 16 contraction stripes

_CACHE = {}
_DEBUG = False


def _build():
    from concourse import bacc
    import concourse.mybir as mybir
    from concourse.tile import TileContext
    from concourse.masks import make_identity

    dt = mybir.dt
    Exp = mybir.ActivationFunctionType.Exp
    nc = bacc.Bacc("TRN2", target_bir_lowering=False, debug=False,
                   num_devices=N_CORES)

    xTd = nc.declare_dram_parameter("xTd", [NCH, 128, ND * IC], dt.bfloat16,
                                    isOutput=False)
    wqkT = nc.declare_dram_parameter("wqkT", [128, NWT * ND * 128],
                                     dt.bfloat16, isOutput=False)
    woT = nc.declare_dram_parameter("woT", [128, ND * D], dt.bfloat16,
                                    isOutput=False)
    cosq = nc.declare_dram_parameter("cosq", [128, S], dt.bfloat16,
                                     isOutput=False)
    sinq = nc.declare_dram_parameter("sinq", [128, S], dt.bfloat16,
                                     isOutput=False)
    coskv = nc.declare_dram_parameter("coskv", [128, S], dt.bfloat16,
                                      isOutput=False)
    sinkv = nc.declare_dram_parameter("sinkv", [128, S], dt.bfloat16,
                                      isOutput=False)
    permM = nc.declare_dram_parameter("permM", [128, 128], dt.bfloat16,
                                      isOutput=False)
    out = nc.declare_dram_parameter("out", [2 * 128, D], dt.float32,
                                    isOutput=True)
    dbg = nc.declare_dram_parameter("dbg", [16, 128, S], dt.float32,
                                    isOutput=True) if _DEBUG else None

    a2a_in = [nc.dram_tensor(f"a2a_in{i}", [N_CORES, 128, 64], dt.bfloat16)
              for i in range(2 * NCH)]
    a2a_out = [nc.dram_tensor(f"a2a_out{i}", [N_CORES, 128, 64], dt.bfloat16)
               for i in range(2 * NCH)]
    cc_sem = nc.alloc_semaphore(name="cc_sem")

    with TileContext(nc) as tc:
        const = tc.alloc_tile_pool(name="const", bufs=1)
        ident = const.tile([128, 128], dt.bfloat16, tag="ident")
        make_identity(nc, ident)
        dmask = const.tile([128, 128], dt.bfloat16, tag="dmask")
        nc.gpsimd.memset(dmask[:], 1.0)
        nc.gpsimd.affine_select(
            out=dmask[:], in_=dmask[:], compare_op=mybir.AluOpType.is_ge,
            fill=0.0, base=0, pattern=[[1, 128]], channel_multiplier=-1)
        perm = const.tile([128, 128], dt.bfloat16, tag="perm")
        nc.gpsimd.dma_start(out=perm[:], in_=permM[:])

        pers = tc.alloc_tile_pool(name="pers", bufs=1)
        wq_sb = pers.tile([128, NWT * ND * 128], dt.bfloat16, tag="wq")
        cs_q = pers.tile([128, S], dt.bfloat16, tag="csq")
        sn_q = pers.tile([128, S], dt.bfloat16, tag="snq")
        cs_kv = pers.tile([128, S], dt.bfloat16, tag="cskv")
        sn_kv = pers.tile([128, S], dt.bfloat16, tag="snkv")
        qkT = [pers.tile([128, S], dt.bfloat16, tag=f"qkT{w}", name=f"qkT{w}")
               for w in range(NWT)]
        kT2 = pers.tile([128, S], dt.bfloat16, tag="kT2")
        v_aug = pers.tile([128, NS * HD1], dt.bfloat16, tag="vaug")
        nc.gpsimd.memset(v_aug[:], 1.0)

        nc.gpsimd.dma_start(out=wq_sb[:], in_=wqkT[:])
        nc.gpsimd.dma_start(out=cs_q[:], in_=cosq[:])
        nc.gpsimd.dma_start(out=sn_q[:], in_=sinq[:])
        nc.gpsimd.dma_start(out=cs_kv[:], in_=coskv[:])
        nc.gpsimd.dma_start(out=sn_kv[:], in_=sinkv[:])

        wo_pool = tc.alloc_tile_pool(name="wo_sb", bufs=1)
        wo_sb = wo_pool.tile([128, ND * D], dt.bfloat16, tag="wo")
        nc.gpsimd.dma_start(out=wo_sb[:], in_=woT[:])

        xpool = tc.alloc_tile_pool(name="xp", bufs=2)
        qkv_ps = tc.alloc_tile_pool(name="qkv_ps", bufs=1, space="PSUM")
        pm_ps = tc.alloc_tile_pool(name="pm_ps", bufs=1, space="PSUM")
        sc_ps = tc.alloc_tile_pool(name="sc_ps", bufs=2, space="PSUM")
        av_ps = tc.alloc_tile_pool(name="av_ps", bufs=1, space="PSUM")
        rope_pool = tc.alloc_tile_pool(name="rope", bufs=2)
        et_pool = tc.alloc_tile_pool(name="et", bufs=3)
        ytmp = tc.alloc_tile_pool(name="ytmp", bufs=2)
        ytf_pool = tc.alloc_tile_pool(name="ytf", bufs=1)

        ytf = {}
        for r in range(N_CORES):
            for p in range(2):
                for half in range(2):
                    ytf[(r, p, half)] = ytf_pool.tile(
                        [128, 128], dt.bfloat16, tag=f"ytf{r}_{p}_{half}",
                        name=f"ytf{r}_{p}_{half}")

        # ---------- emission helpers ----------
        def emit_qkv_chunk_dma(sc):
            xsb = xpool.tile([128, ND * IC], dt.bfloat16, tag="xsb",
                             name="xsb")
            half = ND * IC // 2
            nc.sync.dma_start(out=xsb[:, 0:half], in_=xTd[sc, :, 0:half])
            nc.scalar.dma_start(out=xsb[:, half:], in_=xTd[sc, :, half:])
            return xsb

        def qkv_items(sc, xsb):
            """Yield closures: 3 weight-tile chains (16 matmuls + rope),
            then 4 v transpose-backs."""
            items = []
            state = {}

            def mk_mm(wt, i):
                def f():
                    if i == 0:
                        state[wt] = qkv_ps.tile([128, IC], dt.float32,
                                                tag="qkv", name="qkv")
                    nc.tensor.matmul(
                        state[wt][:],
                        wq_sb[:, (wt * ND + i) * 128:(wt * ND + i + 1) * 128],
                        xsb[:, i * IC:(i + 1) * IC],
                        start=(i == 0), stop=(i == ND - 1))
                return f

            def mk_rope(wt):
                def f():
                    ps = state.pop(wt)
                    cs = cs_q if wt < 2 else cs_kv
                    sn = sn_q if wt < 2 else sn_kv
                    raw = rope_pool.tile([128, IC], dt.bfloat16, tag="raw",
                                         name="raw")
                    nc.vector.tensor_copy(raw[:], ps[:])
                    pm = pm_ps.tile([128, IC], dt.float32, tag="pm",
                                    name="pm")
                    nc.tensor.matmul(pm[:], perm[:], raw[:], start=True,
                                     stop=True)
                    t1 = rope_pool.tile([128, IC], dt.bfloat16, tag="t1",
                                        name="t1")
                    nc.vector.tensor_mul(t1[:], raw[:],
                                         cs[:, sc * IC:(sc + 1) * IC])
                    t2 = rope_pool.tile([128, IC], dt.bfloat16, tag="t2",
                                        name="t2")
                    nc.vector.tensor_mul(t2[:], pm[:],
                                         sn[:, sc * IC:(sc + 1) * IC])
                    nc.vector.tensor_add(qkT[wt][:, sc * IC:(sc + 1) * IC],
                                         t1[:], t2[:])
                    if wt == 2:
                        sl = slice(sc * IC, (sc + 1) * IC)
                        nc.vector.tensor_copy(kT2[0:HD, sl],
                                              qkT[2][0:HD, sl])
                        nc.vector.tensor_copy(kT2[HD:128, sl],
                                              qkT[2][0:HD, sl])
                return f

            def mk_vt(jt):
                def f():
                    pt = pm_ps.tile([128, 1024], dt.bfloat16, tag="pm",
                                    name="pt")
                    nc.tensor.transpose(
                        pt[:, 0:HD],
                        qkT[2][HD:128, jt * 128:(jt + 1) * 128],
                        ident[HD:128, HD:128])
                    nc.vector.tensor_copy(v_aug[:, jt * HD1:jt * HD1 + HD],
                                          pt[:, 0:HD])
                return f

            for wt in range(NWT):
                for i in range(ND):
                    items.append(mk_mm(wt, i))
                items.append(mk_rope(wt))
            for jt in range(4 * sc, 4 * sc + 4):
                items.append(mk_vt(jt))
            return items

        def att_items(c, p):
            """Yield (kind, closure) items for attention chunk c, pair p:
            scores+exp per jt, AV per jt (one jt behind), finalize."""
            items = []
            njt = 4 * c + 4
            ets = {}
            avs = {}

            def mk_sc(jt):
                def f():
                    toff = jt - 4 * c
                    lo = max(toff, 0) * 128
                    w = IC - lo
                    ps_s = sc_ps.tile([128, 2 * IC], dt.float32, tag="sc",
                                      name="sc")
                    for hh in range(2):
                        nc.tensor.matmul(
                            ps_s[:, hh * IC:hh * IC + w],
                            kT2[hh * HD:(hh + 1) * HD,
                                jt * 128:(jt + 1) * 128],
                            qkT[p][hh * HD:(hh + 1) * HD,
                                   c * IC + lo:(c + 1) * IC],
                            start=True, stop=True,
                            tile_position=(hh * HD, 0))
                    et = et_pool.tile([128, 2 * IC], dt.bfloat16, tag="et",
                                      name="et")
                    ets[jt] = et
                    src = ps_s[:, 0:2 * IC].rearrange(
                        "q (h w) -> q h w", h=2)[:, :, 0:w]
                    dst = et[:].rearrange("q (h w) -> q h w", h=2)[:, :, lo:IC]
                    nc.scalar.activation(dst, src, Exp, scale=float(SCALE))
                    if toff >= 0:
                        for hh in range(2):
                            nc.vector.tensor_mul(
                                et[:, hh * IC + lo:hh * IC + lo + 128],
                                et[:, hh * IC + lo:hh * IC + lo + 128],
                                dmask[:])
                return f

            def mk_av(jt):
                def f():
                    if jt == 0:
                        for hh in range(2):
                            avs[hh] = av_ps.tile([128, 4 * HD1], dt.float32,
                                                 tag=f"av{hh}",
                                                 name=f"av{hh}")
                            nc.vector.memset(avs[hh][:], 0.0)
                    et = ets[jt]
                    for hh in range(2):
                        for t in range(4):
                            if jt <= 4 * c + t:
                                nc.tensor.matmul(
                                    avs[hh][:, t * HD1:t * HD1 + HD1],
                                    et[:, hh * IC + t * 128:
                                       hh * IC + (t + 1) * 128],
                                    v_aug[:, jt * HD1:(jt + 1) * HD1],
                                    start=False, stop=(jt == 4 * c + t),
                                    skip_group_check=True)
                    del ets[jt]
                return f

            def fin():
                ys = ytmp.tile([128, IC], dt.bfloat16, tag="ys", name="ys")
                for t in range(4):
                    ypair = ytmp.tile([128, 128], dt.bfloat16, tag="yp",
                                      name="yp")
                    for hh in range(2):
                        base = t * HD1
                        rec = ytmp.tile([128, 1], dt.float32, tag="rec",
                                        name="rec")
                        nc.vector.reciprocal(
                            rec[:], avs[hh][:, base + HD:base + HD + 1])
                        nc.vector.tensor_scalar_mul(
                            ypair[:, hh * HD:(hh + 1) * HD],
                            avs[hh][:, base:base + HD], rec[:])
                    pt = pm_ps.tile([128, 1024], dt.bfloat16, tag="pm",
                                    name="pt")
                    nc.tensor.transpose(pt[:, 0:128], ypair[:], ident[:])
                    nc.vector.tensor_copy(ys[:, t * 128:(t + 1) * 128],
                                          pt[:, 0:128])
                idx = 2 * c + p
                if _DEBUG:
                    nc.gpsimd.dma_start(out=dbg[4 + idx, :, 0:IC],
                                        in_=ys[:])
                idx = 2 * c + p
                for r in range(N_CORES):
                    nc.sync.dma_start(
                        out=a2a_in[idx][r],
                        in_=ys[:, r * 64:(r + 1) * 64])
                nc.gpsimd.collective_compute(
                    "AllToAll", mybir.AluOpType.bypass,
                    replica_groups=[list(range(N_CORES))],
                    ins=[a2a_in[idx][:]],
                    outs=[a2a_out[idx][:]])

            items.append(mk_sc(0))
            for jt in range(1, njt):
                items.append(mk_sc(jt))
                items.append(mk_av(jt - 1))
            items.append(mk_av(njt - 1))
            items.append(fin)
            return items

        def proj_items(half, nchs):
            """Chains for out-projection rows [half*128, half*128+128)."""
            items = []
            state = {}

            def mk_mm(nch, k):
                p, r = divmod(k, N_CORES)
                mt = 2 * r + p

                def f():
                    if k == 0:
                        state[nch] = qkv_ps.tile([128, 512], dt.float32,
                                                 tag="qkv", name="qkv")
                    nc.tensor.matmul(
                        state[nch][:], ytf[(r, p, half)][:],
                        wo_sb[:, mt * D + nch * 512:mt * D + (nch + 1) * 512],
                        start=(k == 0), stop=(k == 15))
                return f

            def mk_store(nch):
                def f():
                    ob = ytmp.tile([128, 512], dt.float32, tag="ob",
                                   name="ob")
                    nc.vector.tensor_copy(ob[:], state.pop(nch)[:])
                    nc.sync.dma_start(
                        out=out[half * 128:(half + 1) * 128,
                                nch * 512:(nch + 1) * 512],
                        in_=ob[:])
                return f

            for nch in nchs:
                for k in range(16):
                    items.append(mk_mm(nch, k))
                items.append(mk_store(nch))
            return items

        def interleave(primary, filler):
            """Emit primary items; spread filler items between them."""
            nf = len(filler)
            np_ = len(primary)
            fi = 0
            for k, item in enumerate(primary):
                item()
                target = (k + 1) * nf // np_
                while fi < target:
                    filler[fi]()
                    fi += 1
            while fi < nf:
                filler[fi]()
                fi += 1

        # ---------- main schedule ----------
        xsb0 = emit_qkv_chunk_dma(0)
        for it in qkv_items(0, xsb0):
            it()
        for sc in range(1, NCH):
            xsb = emit_qkv_chunk_dma(sc)
            att = att_items(sc - 1, 0) + att_items(sc - 1, 1)
            interleave(att, qkv_items(sc, xsb))
        def emit_ytf_dmas(half):
            for cc in range(2):
                c = 2 * half + cc
                for p in range(2):
                    idx = 2 * c + p
                    for r in range(N_CORES):
                        nc.scalar.dma_start(
                            out=ytf[(r, p, half)][:, cc * 64:cc * 64 + 64],
                            in_=a2a_out[idx][r])

        # last chunk attention, overlapped with first projection half
        emit_ytf_dmas(0)
        att = att_items(NCH - 1, 0)
        interleave(att, proj_items(0, [0, 1]))
        att = att_items(NCH - 1, 1)
        interleave(att, proj_items(0, [2, 3]))
        emit_ytf_dmas(1)
        for it in proj_items(1, [0, 1, 2, 3]):
            it()
        if False:
            for p in range(2):
                for r in range(N_CORES):
                    nc.gpsimd.dma_start(
                        out=dbg[12 + p].bitcast(dt.bfloat16)
                        [:, r * 256:(r + 1) * 256],
                        in_=a2a_in[p][r])
                    nc.gpsimd.dma_start(
                        out=dbg[14 + p].bitcast(dt.bfloat16)
                        [:, r * 256:(r + 1) * 256],
                        in_=a2a_out[p][r])
        if False:
            with tc.tile_pool(name="dbgy", bufs=1) as dbgy:
                for p in range(2):
                    for half in range(2):
                        yt_ = dbgy.tile([128, 8 * 128], dt.float32,
                                        tag=f"y{p}{half}", name=f"y{p}{half}")
                        for r in range(N_CORES):
                            nc.vector.tensor_copy(
                                yt_[:, r * 128:(r + 1) * 128],
                                ytf[(r, p, half)][:])
                        nc.sync.dma_start(
                            out=dbg[12 + 2 * p + half, :, 0:1024],
                            in_=yt_[:])
        if _DEBUG:
            with tc.tile_pool(name="dbgp", bufs=1) as dbgp:
                for w in range(3):
                    dt_ = dbgp.tile([128, S], dt.float32, tag=f"dbg{w}",
                                    name=f"dbg{w}")
                    nc.vector.tensor_copy(dt_[:], qkT[w][:])
                    nc.sync.dma_start(out=dbg[w], in_=dt_[:])
                dt_ = dbgp.tile([128, S], dt.float32, tag="dbg3",
                                name="dbg3")
                nc.vector.tensor_copy(dt_[:, 0:NS * HD1], v_aug[:])
                nc.sync.dma_start(out=dbg[3], in_=dt_[:])

        ytf_pool.release()
        ytmp.release()
        et_pool.release()
        rope_pool.release()
        av_ps.release()
        sc_ps.release()
        pm_ps.release()
        qkv_ps.release()
        xpool.release()
        wo_pool.release()
        pers.release()
        const.release()

    nc.compile()
    return nc


def _numpy_reference(x, freqs_cos, freqs_sin, input_pos, wq, wk, wv, wo,
                     k_cache, v_cache):
    B, S_, _ = x.shape
    NKV = 8
    n_rep = NH // NKV

    def rope(t, cos, sin):
        tr = t[..., 0::2]
        ti = t[..., 1::2]
        c = cos[None, :, None, :]
        s = sin[None, :, None, :]
        o = np.stack([tr * c - ti * s, tr * s + ti * c], axis=-1)
        return o.reshape(t.shape)

    q = (x @ wq.T).reshape(B, S_, NH, HD)
    k = (x @ wk.T).reshape(B, S_, NKV, HD)
    v = (x @ wv.T).reshape(B, S_, NKV, HD)
    q = rope(q, freqs_cos, freqs_sin).transpose(0, 2, 1, 3)
    k = rope(k, freqs_cos, freqs_sin).transpose(0, 2, 1, 3)
    v = v.transpose(0, 2, 1, 3)
    k_full = np.array(k_cache)
    v_full = np.array(v_cache)
    k_full[:, :, input_pos] = k
    v_full[:, :, input_pos] = v
    mask = np.tril(np.ones((k_full.shape[2], k_full.shape[2]), bool))[input_pos]
    k_rep = np.repeat(k_full, n_rep, axis=1)
    v_rep = np.repeat(v_full, n_rep, axis=1)
    sc = np.einsum("bhsd,bhtd->bhst", q, k_rep) * SCALE
    sc = np.where(mask[None, None], sc, -np.inf)
    sc = sc - sc.max(axis=-1, keepdims=True)
    e = np.exp(sc)
    attn = e / e.sum(axis=-1, keepdims=True)
    y = np.einsum("bhst,bhtd->bhsd", attn, v_rep)
    y = y.transpose(0, 2, 1, 3).reshape(B, S_, NH * HD)
    return (y @ wo.T).astype(np.float32)


def kernel(x, freqs_cos, freqs_sin, input_pos, wq, wk, wv, wo,
           k_cache, v_cache):
    ipos = np.asarray(input_pos)
    if not np.array_equal(ipos, np.arange(S, dtype=ipos.dtype)):
        return _numpy_reference(np.asarray(x, np.float32),
                                np.asarray(freqs_cos), np.asarray(freqs_sin),
                                ipos, np.asarray(wq), np.asarray(wk),
                                np.asarray(wv), np.asarray(wo),
                                np.asarray(k_cache), np.asarray(v_cache))

    from concourse.bass_utils import run_bass_kernel_spmd

    if "nc" not in _CACHE:
        _CACHE["nc"] = _build()
    nc = _CACHE["nc"]

    bf16 = ml_dtypes.bfloat16
    x2 = np.asarray(x, np.float32)[0].astype(bf16)
    # [sc, p, i, s] with element = xT[128*i + p, 512*sc + s]
    xTd = np.ascontiguousarray(
        x2.T.reshape(ND, 128, NCH, IC).transpose(2, 1, 0, 3)
        .reshape(NCH, 128, ND * IC))

    cos = np.asarray(freqs_cos, np.float32)   # [S, 32]
    sin = np.asarray(freqs_sin, np.float32)
    c64 = np.repeat(cos.T, 2, axis=0)         # [64, S]
    s64 = np.repeat(sin.T, 2, axis=0)
    sgn = np.where(np.arange(HD) % 2 == 0, -1.0, 1.0)[:, None]
    s64 = s64 * sgn
    cosq_h = np.tile(c64, (2, 1)).astype(bf16)
    sinq_h = np.tile(s64, (2, 1)).astype(bf16)
    coskv_h = np.concatenate([c64, np.ones((HD, S))], axis=0).astype(bf16)
    sinkv_h = np.concatenate([s64, np.zeros((HD, S))], axis=0).astype(bf16)

    permM = np.zeros((128, 128), np.float32)
    ii = np.arange(0, 128, 2)
    permM[ii, ii + 1] = 1.0
    permM[ii + 1, ii] = 1.0
    permM = permM.astype(bf16)

    woTf = np.asarray(wo, np.float32).T.astype(bf16)
    woT = np.ascontiguousarray(
        woTf.reshape(ND, 128, D).transpose(1, 0, 2).reshape(128, ND * D))

    wq_f = np.asarray(wq, np.float32)
    wk_f = np.asarray(wk, np.float32)
    wv_f = np.asarray(wv, np.float32)

    in_maps = []
    for c in range(N_CORES):
        wfull = np.concatenate([wq_f[c * QW:(c + 1) * QW],
                                wk_f[c * HD:(c + 1) * HD],
                                wv_f[c * HD:(c + 1) * HD]], axis=0)  # [384, D]
        # [p, wt, i, m] with element = wfull[128*wt + m, 128*i + p]
        wqkT = np.ascontiguousarray(
            wfull.T.reshape(ND, 128, NWT, 128).transpose(1, 2, 0, 3)
            .reshape(128, NWT * ND * 128)).astype(bf16)
        in_maps.append({
            "xTd": xTd, "wqkT": wqkT, "woT": woT,
            "cosq": cosq_h, "sinq": sinq_h,
            "coskv": coskv_h, "sinkv": sinkv_h,
            "permM": permM,
        })

    res = run_bass_kernel_spmd(nc, in_maps, core_ids=list(range(N_CORES)),
                               trace=bool(os.environ.get("KERNEL_TRACE")))
    _CACHE["last_res"] = res
    rows = np.stack([res.results[c]["out"] for c in range(N_CORES)])
    # core r, row 64*c + k  ->  full row 512*c + 64*r + k
    full = rows.reshape(N_CORES, NCH, 64, D).transpose(1, 0, 2, 3)
    return np.ascontiguousarray(full.reshape(S, D))[None].astype(np.float32)
